# revision 7
# baseline (speedup 1.0000x reference)
"""Trainium2 Bass kernel for Conv2d(128->256, 3x3, stride 1, pad 1) on
x(32,128,56,56) fp32, data-parallel over batch across 8 NeuronCores.

Mapping: contraction dim = C_in=128 (SBUF partitions). For each kernel
tap (kh,kw) and each 128-wide output-channel block, one matmul
  psum[co, pix] += w[ci, co].T-free @ x_shifted[ci, pix]
accumulates over the 9 taps into a PSUM bank.

Key point (HW-measured): the PE moving-operand stream runs ~5x faster
when the rhs access pattern is CONTIGUOUS. So pixels are processed in
groups of 8 FULL padded rows (8*58 = 464 <= 512 fp32 PSUM bank limit):
every tap slice is then a flat contiguous window x[(r0+kh)*58+kw :
+464] of the zero-padded 58x58 image (+2 tail floats so the last
group's taps stay in bounds). The two junk columns per row (conv
evaluated at pad-column centers) are stripped by the DVE bias-add,
which reads PSUM strided [8 rows x 56 of 58] and writes the packed
448-wide output tile. Matmul operands use float32r (contiguous f32r
streams ~2 elem/cycle; ~1e-4 scale-relative error vs fp32).
"""
import numpy as np

N_CORES = 8
N_PER_CORE = 4          # 32 images / 8 cores
C_IN, C_OUT, K = 128, 256, 3
H = W = 56
HP = WP = 58            # padded
XFLAT = HP * WP + 2     # +2 so the last tap window stays in bounds
ROWS_PER_GROUP = 8
N_GROUPS = H // ROWS_PER_GROUP  # 7
NFREE = ROWS_PER_GROUP * W      # 448 packed output pixels per group
NWIDE = ROWS_PER_GROUP * WP     # 464 matmul free size (contiguous)

# chosen config (see bench sweeps): contiguous-rhs layout + batched
# stores on the scalar HWDGE queue
MODE = "f32r"
LAYOUT = "r58"

_compiled = {}


def _patch_ldw_opt():
    # walrus ships with --enable-ldw-opt=false hardcoded; enabling the
    # weight-load optimization is verified correct for this kernel and
    # slightly faster. Idempotent process-level patch.
    import concourse.bass_utils as bu

    if getattr(bu.run_command, "_ldw_patched", False):
        return
    orig = bu.run_command

    def patched(argv, **kw):
        argv = ["--enable-ldw-opt=true" if a == "--enable-ldw-opt=false" else a
                for a in argv]
        return orig(argv, **kw)

    patched._ldw_patched = True
    bu.run_command = patched


def _build(reps: int | None = None, mode: str = "f32r",
           store_engine: str = "scalar", store_batch: bool = True,
           probe: str | None = None, wide_n: int | None = None,
           layout: str = "r58"):
    import concourse.bass as bass  # noqa: F401  (engine classes registered)
    import concourse.mybir as mybir
    import concourse.tile as tile
    from concourse import bacc

    _patch_ldw_opt()

    f32 = mybir.dt.float32
    f32r = mybir.dt.float32r
    bf16 = mybir.dt.bfloat16
    x_dt = bf16 if mode in ("bf16", "bf16_out", "xbf16", "xbf16_out") \
        else f32r
    w_dt = bf16 if mode in ("bf16", "bf16_out", "mixed") else f32r
    o_dt = bf16 if mode in ("bf16_out", "xbf16_out") else f32

    if layout == "r64":
        wp, rpg = 64, 7
    else:
        wp, rpg = WP, ROWS_PER_GROUP
    ngr = H // rpg
    xflat_n = HP * wp + 2
    nwide = wide_n if wide_n is not None else rpg * wp

    nc = bacc.Bacc("TRN2", target_bir_lowering=False, debug=False,
                   num_devices=N_CORES)
    x_d = nc.declare_dram_parameter("x", [N_PER_CORE, C_IN, xflat_n], x_dt,
                                    isOutput=False)
    w_d = nc.declare_dram_parameter("w", [C_IN, K * K * C_OUT], w_dt,
                                    isOutput=False)
    b_d = nc.declare_dram_parameter("b", [128, 2], f32, isOutput=False)
    o_d = nc.declare_dram_parameter("o", [N_PER_CORE, 2, 128, H * W], o_dt,
                                    isOutput=True)

    with tile.TileContext(nc) as tc:
        with (
            tc.tile_pool(name="const", bufs=1) as const_pool,
            tc.tile_pool(name="xp", bufs=2) as x_pool,
            tc.tile_pool(name="op", bufs=2) as o_pool,
            tc.tile_pool(name="ps", bufs=8, space="PSUM") as psum_pool,
        ):
            b_sb = const_pool.tile([128, 2], f32, tag="b")
            w_sb = const_pool.tile([C_IN, K * K * C_OUT], w_dt, tag="w")
            x_first = x_pool.tile([C_IN, xflat_n], x_dt, tag="x")
            # interleave weight tap-chunks with first-image row-chunks in the
            # order the first matmul groups consume them (the DMA engine pool
            # drains mostly serially, so emission order is the latency lever);
            # b is only needed by the first DVE bias-add, so it loads third
            nc.sync.dma_start(w_sb[:, 0:256], w_d[:, 0:256])
            nc.sync.dma_start(x_first[:, 0:11 * wp], x_d[0, :, 0:11 * wp])
            nc.sync.dma_start(b_sb[:], b_d[:])
            nc.sync.dma_start(w_sb[:, 256:768], w_d[:, 256:768])
            nc.sync.dma_start(x_first[:, 11 * wp:29 * wp],
                              x_d[0, :, 11 * wp:29 * wp])
            nc.sync.dma_start(w_sb[:, 768:], w_d[:, 768:])
            nc.sync.dma_start(x_first[:, 29 * wp:], x_d[0, :, 29 * wp:])

            store_eng = nc.scalar if store_engine == "scalar" else nc.sync

            def body():
                for n in range(N_PER_CORE):
                    if n == 0 or probe in ("noload", "mmonly"):
                        x_sb = x_first
                    else:
                        x_sb = x_pool.tile([C_IN, xflat_n], x_dt, tag="x")
                        # split the image load so row groups unblock sooner
                        nc.sync.dma_start(x_sb[:, 0:29 * wp],
                                          x_d[n, :, 0:29 * wp])
                        nc.sync.dma_start(x_sb[:, 29 * wp:],
                                          x_d[n, :, 29 * wp:])
                    xflat = x_sb[:]
                    for cob in range(2):
                        o_sb = o_pool.tile([128, H * W], o_dt, tag="o")
                        o4 = o_sb[:].rearrange("p (g r w) -> p g r w",
                                               g=ngr, r=rpg)
                        for rg in range(ngr):
                            ps = psum_pool.tile([128, nwide], f32, tag="ps")
                            idx = 0
                            for kh in range(K):
                                for kw in range(K):
                                    s0 = (rg * rpg + kh) * wp + kw
                                    c0 = ((kh * K + kw) * 2 + cob) * 128
                                    nc.tensor.matmul(
                                        ps[:], w_sb[:, c0:c0 + 128],
                                        xflat[:, s0:s0 + nwide],
                                        start=(idx == 0),
                                        stop=(idx == K * K - 1),
                                    )
                                    idx += 1
                            if probe == "mmonly":
                                continue
                            # bias-add + strip the 2 junk columns per row:
                            # strided PSUM read -> packed SBUF write
                            src = ps[:].rearrange("p (r w) -> p r w",
                                                  w=wp)[:, :, 0:W]
                            nc.vector.tensor_scalar_add(
                                o4[:, rg], src, b_sb[:, cob:cob + 1],
                            )
                            if probe == "nostore":
                                continue
                            if not store_batch:
                                nf = rpg * W
                                store_eng.dma_start(
                                    o_d[n, cob][:, rg * nf:(rg + 1) * nf],
                                    o_sb[:, rg * nf:(rg + 1) * nf],
                                )
                        if store_batch and probe not in ("mmonly",
                                                         "nostore"):
                            # one big store per (n, cob): fewer fixed DMA
                            # costs, longer descriptors
                            store_eng.dma_start(o_d[n, cob][:], o_sb[:])

            if reps is None:
                body()
            else:
                with tc.For_i(0, reps, 1):
                    body()

    nc.compile()
    return nc


def prep_inputs(x, weight, bias, mode: str = "f32r",
                layout: str = "r58"):
    """Host-side layout prep -> per-core input maps."""
    x = np.asarray(x, dtype=np.float32)
    weight = np.asarray(weight, dtype=np.float32)
    bias = np.asarray(bias, dtype=np.float32)

    wp = 64 if layout == "r64" else WP
    xp = np.pad(x, ((0, 0), (0, 0), (1, 1), (1, wp - 1 - W)))
    xp = xp.reshape(N_CORES, N_PER_CORE, C_IN, HP * wp)
    xp = np.pad(xp, ((0, 0), (0, 0), (0, 0), (0, 2)))         # flat +2 tail
    # weight (co, ci, kh, kw) -> (ci, kh, kw, cob, 128) flat [ci, 9*256]
    wr = weight.reshape(2, 128, C_IN, K, K).transpose(2, 3, 4, 0, 1)
    wr = np.ascontiguousarray(wr).reshape(C_IN, K * K * C_OUT)
    br = np.ascontiguousarray(bias.reshape(2, 128).T)          # [128, 2]

    if mode != "f32r":
        import ml_dtypes

        if mode in ("bf16", "bf16_out", "xbf16", "xbf16_out"):
            xp = xp.astype(ml_dtypes.bfloat16)
        if mode in ("bf16", "bf16_out", "mixed"):
            wr = wr.astype(ml_dtypes.bfloat16)

    return [
        {"x": np.ascontiguousarray(xp[c]), "w": wr, "b": br}
        for c in range(N_CORES)
    ]


def kernel(x: np.ndarray, weight: np.ndarray, bias: np.ndarray) -> np.ndarray:
    from concourse.bass_utils import run_bass_kernel_spmd

    if "nc" not in _compiled:
        _compiled["nc"] = _build(mode=MODE, layout=LAYOUT)
    nc = _compiled["nc"]

    in_maps = prep_inputs(x, weight, bias, mode=MODE, layout=LAYOUT)
    res = run_bass_kernel_spmd(nc, in_maps, list(range(N_CORES)))
    out = np.stack([np.asarray(r["o"], dtype=np.float32)
                    for r in res.results])                     # (8,4,2,128,3136)
    out = out.reshape(N_CORES * N_PER_CORE, C_OUT, H, W)
    return out


# revision 8
# speedup vs baseline: 1.2182x; 1.2182x over previous
"""Trainium2 Bass kernel for Conv2d(128->256, 3x3, stride 1, pad 1) on
x(32,128,56,56) fp32, data-parallel over batch across 8 NeuronCores.

Mapping: contraction dim = C_in=128 (SBUF partitions). For each kernel
tap (kh,kw) and each 128-wide output-channel block, one matmul
  psum[co, pix] += w[ci, co].T-free @ x_shifted[ci, pix]
accumulates over the 9 taps into a PSUM bank.

Key point (HW-measured): the PE moving-operand stream runs ~5x faster
when the rhs access pattern is CONTIGUOUS. So pixels are processed in
groups of 8 FULL padded rows (8*58 = 464 <= 512 fp32 PSUM bank limit):
every tap slice is then a flat contiguous window x[(r0+kh)*58+kw :
+464] of the zero-padded 58x58 image (+2 tail floats so the last
group's taps stay in bounds). The two junk columns per row (conv
evaluated at pad-column centers) are stripped by the DVE bias-add,
which reads PSUM strided [8 rows x 56 of 58] and writes the packed
448-wide output tile. Matmul operands use float32r (contiguous f32r
streams ~2 elem/cycle; ~1e-4 scale-relative error vs fp32).
"""
import numpy as np

N_CORES = 8
N_PER_CORE = 4          # 32 images / 8 cores
C_IN, C_OUT, K = 128, 256, 3
H = W = 56
HP = WP = 58            # padded
XFLAT = HP * WP + 2     # +2 so the last tap window stays in bounds
ROWS_PER_GROUP = 8
N_GROUPS = H // ROWS_PER_GROUP  # 7
NFREE = ROWS_PER_GROUP * W      # 448 packed output pixels per group
NWIDE = ROWS_PER_GROUP * WP     # 464 matmul free size (contiguous)

# chosen config (see bench sweeps): contiguous-rhs layout + batched
# stores on the scalar HWDGE queue
MODE = "f32r"
LAYOUT = "r58"

_compiled = {}


def _patch_ldw_opt():
    # walrus ships with --enable-ldw-opt=false hardcoded; enabling the
    # weight-load optimization is verified correct for this kernel and
    # slightly faster. Idempotent process-level patch.
    import concourse.bass_utils as bu

    if getattr(bu.run_command, "_ldw_patched", False):
        return
    orig = bu.run_command

    def patched(argv, **kw):
        argv = ["--enable-ldw-opt=true" if a == "--enable-ldw-opt=false" else a
                for a in argv]
        return orig(argv, **kw)

    patched._ldw_patched = True
    bu.run_command = patched


def _build(reps: int | None = None, mode: str = "f32r",
           store_engine: str = "scalar", store_batch: bool = True,
           probe: str | None = None, wide_n: int | None = None,
           layout: str = "r58", x_bufs: int = 2):
    import concourse.bass as bass  # noqa: F401  (engine classes registered)
    import concourse.mybir as mybir
    import concourse.tile as tile
    from concourse import bacc

    _patch_ldw_opt()

    f32 = mybir.dt.float32
    f32r = mybir.dt.float32r
    bf16 = mybir.dt.bfloat16
    x_dt = bf16 if mode in ("bf16", "bf16_out", "xbf16", "xbf16_out") \
        else f32r
    w_dt = bf16 if mode in ("bf16", "bf16_out", "mixed") else f32r
    o_dt = bf16 if mode in ("bf16_out", "xbf16_out") else f32

    if layout == "r64":
        wp, rpg = 64, 7
    else:
        wp, rpg = WP, ROWS_PER_GROUP
    ngr = H // rpg
    xflat_n = HP * wp + 2
    nwide = wide_n if wide_n is not None else rpg * wp

    nc = bacc.Bacc("TRN2", target_bir_lowering=False, debug=False,
                   num_devices=N_CORES)
    x_d = nc.declare_dram_parameter("x", [N_PER_CORE, C_IN, xflat_n], x_dt,
                                    isOutput=False)
    w_d = nc.declare_dram_parameter("w", [C_IN, K * K * C_OUT], w_dt,
                                    isOutput=False)
    b_d = nc.declare_dram_parameter("b", [128, 2], f32, isOutput=False)
    o_d = nc.declare_dram_parameter("o", [N_PER_CORE, 2, 128, H * W], o_dt,
                                    isOutput=True)

    with tile.TileContext(nc) as tc:
        with (
            tc.tile_pool(name="const", bufs=1) as const_pool,
            tc.tile_pool(name="xp", bufs=x_bufs) as x_pool,
            tc.tile_pool(name="op", bufs=2) as o_pool,
            tc.tile_pool(name="ps", bufs=8, space="PSUM") as psum_pool,
        ):
            b_sb = const_pool.tile([128, 2], f32, tag="b")
            w_sb = const_pool.tile([C_IN, K * K * C_OUT], w_dt, tag="w")
            x_first = x_pool.tile([C_IN, xflat_n], x_dt, tag="x")
            # interleave weight tap-chunks with first-image row-chunks in the
            # order the first matmul groups consume them (the DMA engine pool
            # drains mostly serially, so emission order is the latency lever);
            # b is only needed by the first DVE bias-add, so it loads third
            nc.sync.dma_start(w_sb[:, 0:256], w_d[:, 0:256])
            nc.sync.dma_start(x_first[:, 0:11 * wp], x_d[0, :, 0:11 * wp])
            nc.sync.dma_start(b_sb[:], b_d[:])
            nc.sync.dma_start(w_sb[:, 256:768], w_d[:, 256:768])
            nc.sync.dma_start(x_first[:, 11 * wp:29 * wp],
                              x_d[0, :, 11 * wp:29 * wp])
            nc.sync.dma_start(w_sb[:, 768:], w_d[:, 768:])
            nc.sync.dma_start(x_first[:, 29 * wp:], x_d[0, :, 29 * wp:])

            store_eng = nc.scalar if store_engine == "scalar" else nc.sync

            def body():
                for n in range(N_PER_CORE):
                    if n == 0 or probe in ("noload", "mmonly"):
                        x_sb = x_first
                    else:
                        x_sb = x_pool.tile([C_IN, xflat_n], x_dt, tag="x")
                        # split the image load so row groups unblock sooner
                        nc.sync.dma_start(x_sb[:, 0:29 * wp],
                                          x_d[n, :, 0:29 * wp])
                        nc.sync.dma_start(x_sb[:, 29 * wp:],
                                          x_d[n, :, 29 * wp:])
                    xflat = x_sb[:]
                    for cob in range(2):
                        o_sb = o_pool.tile([128, H * W], o_dt, tag="o")
                        o4 = o_sb[:].rearrange("p (g r w) -> p g r w",
                                               g=ngr, r=rpg)
                        for rg in range(ngr):
                            ps = psum_pool.tile([128, nwide], f32, tag="ps")
                            idx = 0
                            for kh in range(K):
                                for kw in range(K):
                                    s0 = (rg * rpg + kh) * wp + kw
                                    c0 = ((kh * K + kw) * 2 + cob) * 128
                                    nc.tensor.matmul(
                                        ps[:], w_sb[:, c0:c0 + 128],
                                        xflat[:, s0:s0 + nwide],
                                        start=(idx == 0),
                                        stop=(idx == K * K - 1),
                                    )
                                    idx += 1
                            if probe == "mmonly":
                                continue
                            # bias-add + strip the 2 junk columns per row:
                            # strided PSUM read -> packed SBUF write
                            src = ps[:].rearrange("p (r w) -> p r w",
                                                  w=wp)[:, :, 0:W]
                            nc.vector.tensor_scalar_add(
                                o4[:, rg], src, b_sb[:, cob:cob + 1],
                            )
                            if probe == "nostore":
                                continue
                            if not store_batch:
                                nf = rpg * W
                                store_eng.dma_start(
                                    o_d[n, cob][:, rg * nf:(rg + 1) * nf],
                                    o_sb[:, rg * nf:(rg + 1) * nf],
                                )
                        if store_batch and probe not in ("mmonly",
                                                         "nostore"):
                            # one big store per (n, cob): fewer fixed DMA
                            # costs, longer descriptors
                            store_eng.dma_start(o_d[n, cob][:], o_sb[:])

            if reps is None:
                body()
            else:
                with tc.For_i(0, reps, 1):
                    body()

    nc.compile()
    return nc


def prep_inputs(x, weight, bias, mode: str = "f32r",
                layout: str = "r58"):
    """Host-side layout prep -> per-core input maps."""
    x = np.asarray(x, dtype=np.float32)
    weight = np.asarray(weight, dtype=np.float32)
    bias = np.asarray(bias, dtype=np.float32)

    wp = 64 if layout == "r64" else WP
    xp = np.pad(x, ((0, 0), (0, 0), (1, 1), (1, wp - 1 - W)))
    xp = xp.reshape(N_CORES, N_PER_CORE, C_IN, HP * wp)
    xp = np.pad(xp, ((0, 0), (0, 0), (0, 0), (0, 2)))         # flat +2 tail
    # weight (co, ci, kh, kw) -> (ci, kh, kw, cob, 128) flat [ci, 9*256]
    wr = weight.reshape(2, 128, C_IN, K, K).transpose(2, 3, 4, 0, 1)
    wr = np.ascontiguousarray(wr).reshape(C_IN, K * K * C_OUT)
    br = np.ascontiguousarray(bias.reshape(2, 128).T)          # [128, 2]

    if mode != "f32r":
        import ml_dtypes

        if mode in ("bf16", "bf16_out", "xbf16", "xbf16_out"):
            xp = xp.astype(ml_dtypes.bfloat16)
        if mode in ("bf16", "bf16_out", "mixed"):
            wr = wr.astype(ml_dtypes.bfloat16)

    return [
        {"x": np.ascontiguousarray(xp[c]), "w": wr, "b": br}
        for c in range(N_CORES)
    ]


def kernel(x: np.ndarray, weight: np.ndarray, bias: np.ndarray) -> np.ndarray:
    from concourse.bass_utils import run_bass_kernel_spmd

    if "nc" not in _compiled:
        _compiled["nc"] = _build(mode=MODE, layout=LAYOUT)
    nc = _compiled["nc"]

    in_maps = prep_inputs(x, weight, bias, mode=MODE, layout=LAYOUT)
    res = run_bass_kernel_spmd(nc, in_maps, list(range(N_CORES)))
    out = np.stack([np.asarray(r["o"], dtype=np.float32)
                    for r in res.results])                     # (8,4,2,128,3136)
    out = out.reshape(N_CORES * N_PER_CORE, C_OUT, H, W)
    return out
